# revision 1
# baseline (speedup 1.0000x reference)
"""Trainium2 Bass kernel for nn_Attention (2-batch, 16-head, n=2048, d=64 causal
attention with LayerNorm-projected l2-normalized q/k, relative position bias,
and output projection), SPMD across 8 NeuronCores.

Sharding: launch A tensor-parallels the 16 heads (2 heads per core, both
batches on every core) and emits transposed attention outputs; launch B
row-shards the final @ Wo matmul across the 8 cores.

Matmuls run as bf16 (projections, stats, sim, attn@v) or float32r
(reduced-precision fp32, ~tf32 accuracy, full PE rate at N>=256; used where
rounding matters).  LayerNorm is folded into the projections: gamma folds into
the weights, the mean subtraction becomes a rank-1 matmul accumulation, and
rstd cancels in the q/k l2norm (applied to v as a cheap broadcast multiply).
Attention is computed in transposed layout simT[j, i] so the softmax
denominator falls out of the attn@v matmul via an appended ones-column on v
(row 64 of the output carries the denominators; launch B normalizes), and
causal masking is an affine_select on the diagonal bias tiles.
"""

import numpy as np

HEADS = 16
DH = 64
B = 2
N = 2048
DIM = 1024
EH = 128          # per-core slice of the inner dim (2 heads x 64)
NCORES = 8
IC = 512          # i-chunk width
NIC = N // IC     # 4 i-chunks
JT = 128          # j-tile width
NJT = N // JT     # 16 j-tiles
NCT = DIM // 128  # 8 contraction tiles
LN_EPS = 1e-5
NEG = -1e30

_cache = {}


def _build_launch_a():
    import concourse.bass as bass
    import concourse.tile as tile
    from concourse import bacc, mybir
    from concourse.masks import make_identity

    F32 = mybir.dt.float32
    F32R = mybir.dt.float32r
    BF16 = mybir.dt.bfloat16
    AF = mybir.ActivationFunctionType
    nc = bacc.Bacc(None)
    xT_d = nc.declare_dram_parameter("xT", [B, DIM, N], BF16, isOutput=False)
    rpbT_d = nc.declare_dram_parameter("rpbT", [2, N, N], F32, isOutput=False)
    wq_d = nc.declare_dram_parameter("wq", [DIM, EH], F32, isOutput=False)
    wk_d = nc.declare_dram_parameter("wk", [DIM, EH], F32, isOutput=False)
    wv_d = nc.declare_dram_parameter("wv", [DIM, EH], F32, isOutput=False)
    gamma_d = nc.declare_dram_parameter("gamma", [DIM], F32, isOutput=False)
    qs2_d = nc.declare_dram_parameter("qs2", [EH], F32, isOutput=False)
    ks2_d = nc.declare_dram_parameter("ks2", [EH], F32, isOutput=False)
    kb_d = nc.declare_dram_parameter("kb", [B, N], F32, isOutput=False)
    at_d = nc.declare_dram_parameter("at_out", [B, 2, 65, N], F32, isOutput=True)

    with tile.TileContext(nc) as tc:
        import contextlib
        with contextlib.ExitStack() as ctx:
            pers = ctx.enter_context(tc.tile_pool(name="pers", bufs=1))

            # ---------- constants ----------
            onescol_f = pers.tile([128, 1], F32, tag="onescol_f")
            nc.vector.memset(onescol_f, 1.0)
            onescol = pers.tile([128, 1], F32R, tag="onescol")
            nc.vector.tensor_copy(out=onescol, in_=onescol_f)
            onescol_bf = pers.tile([128, 1], BF16, tag="onescol_bf")
            nc.vector.tensor_copy(out=onescol_bf, in_=onescol_f)
            ones_row_bf = pers.tile([1, 128], BF16, tag="ones_row_bf")
            ident_bf = pers.tile([128, 128], BF16, tag="ident_bf")
            row_f = pers.tile([1, 128], F32, tag="row_f")
            nc.vector.memset(row_f, 1.0)
            ones_row = pers.tile([1, 128], F32R, tag="ones_row")
            nc.vector.tensor_copy(out=ones_row, in_=row_f)
            nc.vector.tensor_copy(out=ones_row_bf, in_=row_f)
            invn_f = pers.tile([1, 128], F32, tag="invn_f")
            nc.vector.memset(invn_f, 1.0 / DIM)
            invn_row = pers.tile([1, 128], F32R, tag="invn_row")
            nc.vector.tensor_copy(out=invn_row, in_=invn_f)
            o2_f = pers.tile([128, 2], F32, tag="o2_f")
            nc.vector.memset(o2_f, 0.0)
            nc.vector.memset(o2_f[0:64, 0:1], 1.0)
            nc.vector.memset(o2_f[64:128, 1:2], 1.0)
            ones2blk = pers.tile([128, 2], F32R, tag="ones2blk")
            nc.vector.tensor_copy(out=ones2blk, in_=o2_f)
            ident = pers.tile([128, 128], F32, tag="ident")
            make_identity(nc, ident)
            nc.vector.tensor_copy(out=ident_bf, in_=ident)
            eps128 = pers.tile([128, 1], F32, tag="eps128")
            nc.vector.memset(eps128, LN_EPS)
            eps2 = pers.tile([2, 1], F32, tag="eps2")
            nc.vector.memset(eps2, 1e-24)

            # scale rows -> block-diag [2, 128] (qs2blk[h, e] = qs2[e] iff head(e)==h)
            qsb_f = pers.tile([2, 128], F32, tag="qsb_f")
            nc.vector.memset(qsb_f, 0.0)
            nc.sync.dma_start(out=qsb_f[0:1, 0:64], in_=qs2_d.ap()[0:64].unsqueeze(0))
            nc.sync.dma_start(out=qsb_f[1:2, 64:128], in_=qs2_d.ap()[64:128].unsqueeze(0))
            qs2blk = pers.tile([2, 128], F32R, tag="qs2blk")
            nc.vector.tensor_copy(out=qs2blk, in_=qsb_f)
            ksb_f = pers.tile([2, 128], F32, tag="ksb_f")
            nc.vector.memset(ksb_f, 0.0)
            nc.sync.dma_start(out=ksb_f[0:1, 0:64], in_=ks2_d.ap()[0:64].unsqueeze(0))
            nc.sync.dma_start(out=ksb_f[1:2, 64:128], in_=ks2_d.ap()[64:128].unsqueeze(0))
            ks2blk = pers.tile([2, 128], F32R, tag="ks2blk")
            nc.vector.tensor_copy(out=ks2blk, in_=ksb_f)

            gam_sb = pers.tile([128, NCT], F32, tag="gam")
            nc.sync.dma_start(out=gam_sb, in_=gamma_d.ap().rearrange("(t p) -> p t", p=128))
            kbT = pers.tile([128, B, NJT], F32, tag="kbT")
            nc.sync.dma_start(out=kbT, in_=kb_d.ap().rearrange("b (t p) -> p b t", p=128))

            # ---------- weights: load, fold gamma, round to f32r ----------
            wps = {}
            css = {}
            with tc.tile_pool(name="wload", bufs=2) as wload, \
                 tc.tile_pool(name="cs_ps", bufs=1, space="PSUM") as cs_ps:
                for nm, wd in (("q", wq_d), ("k", wk_d), ("v", wv_d)):
                    wraw = wload.tile([128, NCT, EH], F32, tag="wraw")
                    nc.sync.dma_start(out=wraw, in_=wd.ap().rearrange("(t p) e -> p t e", p=128))
                    wp = pers.tile([128, NCT, EH], BF16, tag=f"w{nm}p")
                    for ct in range(NCT):
                        nc.vector.tensor_scalar_mul(
                            out=wp[:, ct, :], in0=wraw[:, ct, :], scalar1=gam_sb[:, ct:ct + 1])
                    cs = cs_ps.tile([1, EH], F32, tag="cs")
                    for ct in range(NCT):
                        nc.tensor.matmul(cs, onescol_bf, wp[:, ct, :],
                                         start=(ct == 0), stop=(ct == NCT - 1))
                    cs_sb = pers.tile([1, EH], BF16, tag=f"cs{nm}")
                    nc.vector.tensor_copy(out=cs_sb, in_=cs)
                    wps[nm] = wp
                    css[nm] = cs_sb

            # ---------- persistent per-batch products ----------
            qhat = [pers.tile([128, N], BF16, tag=f"qhat{b}", name=f"qhat{b}") for b in range(B)]
            khat = [pers.tile([128, N], BF16, tag=f"khat{b}", name=f"khat{b}") for b in range(B)]
            v_all = [pers.tile([128, NJT, 130], BF16, tag=f"vall{b}", name=f"vall{b}") for b in range(B)]

            for b in range(B):
                for jt in range(NJT):
                    nc.vector.tensor_copy(out=v_all[b][:, jt, 64:65], in_=onescol_f)
                    nc.vector.tensor_copy(out=v_all[b][:, jt, 129:130], in_=onescol_f)

            # ================= phase 1: LN stats + projections =================
            with tc.tile_pool(name="p1", bufs=2) as p1, \
                 tc.tile_pool(name="p1b", bufs=3) as p1b, \
                 tc.tile_pool(name="xr_pool", bufs=1) as xr_pool, \
                 tc.tile_pool(name="st_ps", bufs=1, space="PSUM") as st_ps, \
                 tc.tile_pool(name="pp_ps", bufs=4, space="PSUM") as pp_ps, \
                 tc.tile_pool(name="bc_ps", bufs=2, space="PSUM") as bc_ps:
                for b in range(B):
                    xr = xr_pool.tile([128, NCT, N], BF16, tag="xr")
                    for half in range(2):
                        hs = slice(half * (NCT // 2), (half + 1) * (NCT // 2))
                        nc.sync.dma_start(
                            out=xr[:, hs, :],
                            in_=xT_d.ap()[b, half * 512:(half + 1) * 512, :].rearrange(
                                "(t p) n -> p t n", p=128))

                    numu = pers.tile([1, N], BF16, tag=f"numu{b}")
                    for ic in range(NIC):
                        isl = slice(ic * IC, (ic + 1) * IC)
                        # --- stats ---
                        sx = st_ps.tile([1, IC], F32, tag="sx")
                        sxx = st_ps.tile([1, IC], F32, tag="sxx")
                        for ct in range(NCT):
                            nc.tensor.matmul(sx, onescol_bf, xr[:, ct, isl],
                                             start=(ct == 0), stop=(ct == NCT - 1))
                        for ct in range(NCT):
                            x2 = p1b.tile([128, IC], BF16, tag="tmpb")
                            nc.vector.tensor_mul(x2, xr[:, ct, isl], xr[:, ct, isl])
                            nc.tensor.matmul(sxx, onescol_bf, x2,
                                             start=(ct == 0), stop=(ct == NCT - 1))
                        nc.scalar.mul(out=numu[:, isl], in_=sx, mul=-1.0 / DIM)
                        sxx_sb = p1b.tile([1, IC], F32R, tag="rowtmp")
                        nc.vector.tensor_copy(out=sxx_sb, in_=sxx)
                        # broadcast stats to 128 partitions via K=1 matmuls
                        mub = bc_ps.tile([128, IC], F32, tag="bc")
                        nc.tensor.matmul(mub, ones_row_bf, numu[:, isl], start=True, stop=True)
                        sxxb = bc_ps.tile([128, IC], F32, tag="bc")
                        nc.tensor.matmul(sxxb, invn_row, sxx_sb, start=True, stop=True)
                        mu2 = p1b.tile([128, IC], F32, tag="tmp")
                        nc.scalar.activation(out=mu2, in_=mub, func=AF.Square)
                        var = p1b.tile([128, IC], F32, tag="tmp")
                        nc.vector.tensor_sub(var, sxxb, mu2)
                        std = p1b.tile([128, IC], F32, tag="tmp")
                        nc.scalar.activation(out=std, in_=var, func=AF.Sqrt, bias=eps128)
                        rstd = p1b.tile([128, IC], F32, tag="rstd", bufs=2)
                        nc.vector.reciprocal_approx_fast(out=rstd, in_=std)

                        # --- q/k projections + l2norm ---
                        for nm, hat, sblk, sq_scale in (
                            ("q", qhat[b], qs2blk, 1.0 / 64.0),
                            ("k", khat[b], ks2blk, 1.0),
                        ):
                            pp = pp_ps.tile([128, IC], F32, tag="proj")
                            for ct in range(NCT):
                                nc.tensor.matmul(pp, wps[nm][:, ct, :], xr[:, ct, isl],
                                                 start=(ct == 0), stop=False)
                            nc.tensor.matmul(pp, css[nm], numu[:, isl], start=False, stop=True)
                            sq = p1b.tile([128, IC], F32R, tag="tmp")
                            nc.scalar.activation(out=sq, in_=pp, func=AF.Square)
                            ssq = bc_ps.tile([2, IC], F32, tag="bc")
                            nc.tensor.matmul(ssq, ones2blk, sq, start=True, stop=True)
                            rt = p1b.tile([2, IC], F32, tag="rowtmp")
                            nc.scalar.activation(out=rt, in_=ssq, func=AF.Sqrt,
                                                 bias=eps2, scale=sq_scale)
                            rn_f = p1b.tile([2, IC], F32, tag="rowtmp")
                            nc.vector.reciprocal_approx_fast(out=rn_f, in_=rt)
                            rn = p1b.tile([2, IC], F32R, tag="rowtmp")
                            nc.vector.tensor_copy(out=rn, in_=rn_f)
                            sr = bc_ps.tile([128, IC], F32, tag="bc")
                            nc.tensor.matmul(sr, sblk, rn, start=True, stop=True)
                            sr_sb = p1b.tile([128, IC], F32, tag="srsb")
                            nc.vector.tensor_copy(out=sr_sb, in_=sr)
                            nc.vector.tensor_mul(hat[:, isl], pp, sr_sb)

                        # --- v projection (rstd applied), transpose to [j, e] ---
                        vp = pp_ps.tile([128, IC], F32, tag="proj")
                        for ct in range(NCT):
                            nc.tensor.matmul(vp, wps["v"][:, ct, :], xr[:, ct, isl],
                                             start=(ct == 0), stop=False)
                        nc.tensor.matmul(vp, css["v"], numu[:, isl], start=False, stop=True)
                        vsc = p1b.tile([128, IC], BF16, tag="tmpb")
                        nc.vector.tensor_mul(vsc, vp, rstd)
                        for k in range(IC // 128):
                            jt = ic * (IC // 128) + k
                            vt = bc_ps.tile([128, 128], BF16, tag="bc")
                            nc.tensor.transpose(vt, vsc[:, k * 128:(k + 1) * 128], ident_bf)
                            nc.vector.tensor_copy(out=v_all[b][:, jt, 0:64], in_=vt[:, 0:64])
                            nc.vector.tensor_copy(out=v_all[b][:, jt, 65:129], in_=vt[:, 64:128])

            # ================= phase 2: attention =================
            with tc.tile_pool(name="rp_pool", bufs=4) as rp_pool, \
                 tc.tile_pool(name="rpd_pool", bufs=8) as rpd_pool, \
                 tc.tile_pool(name="es_pool", bufs=5) as es_pool, \
                 tc.tile_pool(name="E_pool", bufs=4) as E_pool, \
                 tc.tile_pool(name="at_pool", bufs=2) as at_pool, \
                 tc.tile_pool(name="sm_ps", bufs=4, space="PSUM") as sm_ps, \
                 tc.tile_pool(name="av_ps", bufs=2, space="PSUM") as av_ps:
                for ic in range(NIC):
                    isl = slice(ic * IC, (ic + 1) * IC)
                    jmax = (IC // 128) * (ic + 1)
                    rps = []
                    rpds = []
                    for h in range(2):
                        rp = rp_pool.tile([128, NJT, IC], F32, tag="rp", name=f"rp{h}")
                        nc.sync.dma_start(
                            out=rp[:, 0:jmax, :],
                            in_=rpbT_d.ap()[h, 0:jmax * 128, isl].rearrange(
                                "(t p) i -> p t i", p=128))
                        rpd = []
                        for k in range(IC // 128):
                            jt = jmax - (IC // 128) + k
                            rd = rpd_pool.tile([128, IC], F32, tag="rpd", name=f"rpd{h}{k}")
                            nc.gpsimd.affine_select(
                                out=rd, in_=rp[:, jt, :],
                                compare_op=mybir.AluOpType.is_ge,
                                fill=NEG, base=-128 * k, channel_multiplier=-1,
                                pattern=[[1, IC]])
                            rpd.append(rd)
                        rps.append(rp)
                        rpds.append(rpd)
                    for b in range(B):
                        avs = [av_ps.tile([65, IC], F32, tag=f"av{h}", name=f"av{h}")
                               for h in range(2)]
                        for jg in range(0, jmax, 2):
                            sps = {}
                            for jt in range(jg, min(jg + 2, jmax)):
                                for h in range(2):
                                    dsl = slice(64 * h, 64 * h + 64)
                                    sp = sm_ps.tile([128, IC], F32, tag="sim", name="sp")
                                    nc.tensor.matmul(
                                        sp, khat[b][dsl, jt * 128:(jt + 1) * 128],
                                        qhat[b][dsl, isl], start=True, stop=True)
                                    sps[jt, h] = sp
                            for jt in range(jg, min(jg + 2, jmax)):
                                for h in range(2):
                                    diag_k = jt - (jmax - (IC // 128))
                                    bias_tile = rpds[h][diag_k] if diag_k >= 0 else rps[h][:, jt, :]
                                    es = es_pool.tile([128, IC], F32, tag="es")
                                    nc.vector.tensor_add(es, sps[jt, h], bias_tile)
                                    E = E_pool.tile([128, IC], BF16, tag="E")
                                    nc.scalar.activation(out=E, in_=es, func=AF.Exp,
                                                         bias=kbT[:, b, jt:jt + 1])
                                    nc.tensor.matmul(
                                        avs[h], v_all[b][:, jt, 65 * h:65 * h + 65], E,
                                        start=(jt == 0), stop=(jt == jmax - 1))
                        for h in range(2):
                            stg = at_pool.tile([65, IC], F32, tag="stg")
                            nc.vector.tensor_copy(out=stg, in_=avs[h][0:65, :])
                            nc.sync.dma_start(out=at_d.ap()[b, h, :, isl], in_=stg)
    nc.compile()
    return nc


def _build_launch_b():
    import concourse.bass as bass
    import concourse.tile as tile
    from concourse import bacc, mybir

    F32 = mybir.dt.float32
    F32R = mybir.dt.float32r
    BF16 = mybir.dt.bfloat16

    nc = bacc.Bacc(None)
    at_d = nc.declare_dram_parameter("a_t", [DIM, IC], F32, isOutput=False)
    s_d = nc.declare_dram_parameter("s_slice", [HEADS, IC], F32, isOutput=False)
    sel_d = nc.declare_dram_parameter("sel", [NCT, HEADS, 128], F32, isOutput=False)
    wo_d = nc.declare_dram_parameter("wo", [DIM, DIM], F32, isOutput=False)
    out_d = nc.declare_dram_parameter("out_rows", [IC, DIM], F32, isOutput=True)

    with tile.TileContext(nc) as tc:
        with tc.tile_pool(name="sb", bufs=1) as sb, \
             tc.tile_pool(name="wl", bufs=2) as wl, \
             tc.tile_pool(name="ob", bufs=2) as ob, \
             tc.tile_pool(name="rb_ps", bufs=2, space="PSUM") as rb_ps, \
             tc.tile_pool(name="ps", bufs=2, space="PSUM") as ps:
            a_sb = sb.tile([128, NCT, IC], F32, tag="a")
            for half in range(2):
                hs = slice(half * (NCT // 2), (half + 1) * (NCT // 2))
                nc.sync.dma_start(
                    out=a_sb[:, hs, :],
                    in_=at_d.ap()[half * 512:(half + 1) * 512, :].rearrange(
                        "(t p) i -> p t i", p=128))
            s_sb = sb.tile([HEADS, IC], F32, tag="s")
            nc.sync.dma_start(out=s_sb, in_=s_d.ap())
            sel_sb = sb.tile([HEADS, NCT, 128], F32, tag="sel")
            nc.sync.dma_start(out=sel_sb, in_=sel_d.ap().rearrange("t h p -> h t p"))
            rs_f = sb.tile([HEADS, IC], F32, tag="rs_f")
            nc.vector.reciprocal_approx_fast(out=rs_f, in_=s_sb)
            rs_r = sb.tile([HEADS, IC], F32R, tag="rs_r")
            nc.vector.tensor_copy(out=rs_r, in_=rs_f)
            wo_r = sb.tile([128, NCT, DIM], F32R, tag="wo")
            for ct in range(NCT):
                wr = wl.tile([128, DIM], F32, tag="wr")
                nc.sync.dma_start(out=wr, in_=wo_d.ap()[ct * 128:(ct + 1) * 128, :])
                nc.vector.tensor_copy(out=wo_r[:, ct, :], in_=wr)
            # normalized bf16 activations: a_n[c, i] = a[c, i] / s[head(c), i]
            a_n = sb.tile([128, NCT, IC], F32R, tag="a_n")
            for ct in range(NCT):
                selr = wl.tile([HEADS, 128], F32R, tag="selr")
                nc.vector.tensor_copy(out=selr, in_=sel_sb[:, ct, :])
                rsb = rb_ps.tile([128, IC], F32, tag="rsb")
                nc.tensor.matmul(rsb, selr, rs_r, start=True, stop=True)
                nc.vector.tensor_mul(a_n[:, ct, :], rsb, a_sb[:, ct, :])
            for m in range(IC // 128):
                osb = ob.tile([128, DIM], F32, tag="osb")
                for oc in range(2):
                    pp = ps.tile([128, 512], F32, tag="pp")
                    for ct in range(NCT):
                        nc.tensor.matmul(
                            pp, a_n[:, ct, m * 128:(m + 1) * 128],
                            wo_r[:, ct, oc * 512:(oc + 1) * 512],
                            start=(ct == 0), stop=(ct == NCT - 1))
                    nc.vector.tensor_copy(out=osb[:, oc * 512:(oc + 1) * 512], in_=pp)
                nc.sync.dma_start(out=out_d.ap()[m * 128:(m + 1) * 128, :], in_=osb)

    nc.compile()
    return nc


PROFILE = {"enabled": False, "a_ns": None, "b_ns": None}


def _install_profile_hook():
    """Register the axon NTFF profile hook (the image's antenv lacks
    axon_hooks, so run_bass_kernel_spmd(trace=True) would silently skip
    tracing).  Replicates trn_boot's ctypes recipe."""
    import sys, types, ctypes, contextlib

    if "antenv.axon_hooks" in sys.modules:
        return
    lib = ctypes.CDLL("/opt/axon/libaxon_pjrt.so")
    if not hasattr(lib, "axon_start_nrt_profile"):
        return
    lib.axon_start_nrt_profile.argtypes = [ctypes.POINTER(ctypes.c_int64), ctypes.c_size_t]
    lib.axon_start_nrt_profile.restype = ctypes.c_int64
    lib.axon_stop_nrt_profile.argtypes = [ctypes.c_char_p]
    lib.axon_stop_nrt_profile.restype = ctypes.c_int64

    @contextlib.contextmanager
    def _hook(output_dir, device_ids):
        import jax
        jax.devices()
        if device_ids:
            ids = (ctypes.c_int64 * len(device_ids))(*device_ids)
            rc = lib.axon_start_nrt_profile(ids, len(device_ids))
        else:
            rc = lib.axon_start_nrt_profile(None, 0)
        if rc != 0:
            raise RuntimeError(f"axon_start_nrt_profile rc={rc}")
        try:
            yield
        finally:
            n = lib.axon_stop_nrt_profile(str(output_dir).encode())
            print(f"profile: {n} file(s) written to {output_dir}")

    mod = types.ModuleType("antenv.axon_hooks")
    mod.get_axon_ntff_profile_hook = lambda: _hook
    mod.set_axon_ntff_profile_hook = lambda h: None
    sys.modules["antenv.axon_hooks"] = mod

    # avoid the S3 artifact upload inside the trace path
    from concourse import bass_utils
    bass_utils.upload_artifacts = lambda tmpdir: ""


def kernel(x, gamma, Wq, Wkv, q_scale, k_scale, Wo, rel_pos_bias, mask):
    from concourse.bass_utils import run_bass_kernel_spmd

    x = np.ascontiguousarray(np.asarray(x, dtype=np.float32))
    gamma = np.asarray(gamma, dtype=np.float32)
    Wq = np.asarray(Wq, dtype=np.float32)
    Wkv = np.asarray(Wkv, dtype=np.float32)
    q_scale = np.asarray(q_scale, dtype=np.float32)
    k_scale = np.asarray(k_scale, dtype=np.float32)
    Wo = np.ascontiguousarray(np.asarray(Wo, dtype=np.float32))
    rel_pos_bias = np.asarray(rel_pos_bias, dtype=np.float32)
    mask = np.asarray(mask)

    if PROFILE["enabled"]:
        _install_profile_hook()
    if "a" not in _cache:
        _cache["a"] = _build_launch_a()
    if "b" not in _cache:
        _cache["b"] = _build_launch_b()

    import ml_dtypes
    xT = np.ascontiguousarray(x.transpose(0, 2, 1)).astype(ml_dtypes.bfloat16)
    kb = np.where(mask, 0.0, NEG).astype(np.float32)
    qs2 = np.tile(q_scale, 2).astype(np.float32)
    ks2 = np.tile(k_scale, 2).astype(np.float32)

    in_maps_a = []
    for c in range(NCORES):
        es = slice(EH * c, EH * (c + 1))
        in_maps_a.append({
            "xT": xT,
            "rpbT": np.ascontiguousarray(rel_pos_bias[2 * c:2 * c + 2].transpose(0, 2, 1)),
            "wq": np.ascontiguousarray(Wq[:, es]),
            "wk": np.ascontiguousarray(Wkv[:, :DIM][:, es]),
            "wv": np.ascontiguousarray(Wkv[:, DIM:][:, es]),
            "gamma": gamma, "qs2": qs2, "ks2": ks2, "kb": kb,
        })
    res_a = run_bass_kernel_spmd(_cache["a"], in_maps_a, list(range(NCORES)),
                                 trace=PROFILE["enabled"])
    if PROFILE["enabled"]:
        PROFILE["a_ns"] = res_a.exec_time_ns

    AT = np.empty((B, DIM, N), np.float32)
    S = np.empty((B, HEADS, N), np.float32)
    for c in range(NCORES):
        ao = res_a.results[c]["at_out"]            # [B, 2, 65, N]
        for h in range(2):
            AT[:, EH * c + 64 * h:EH * c + 64 * h + 64, :] = ao[:, h, 0:64, :]
            S[:, 2 * c + h, :] = ao[:, h, 64, :]

    sel = np.zeros((NCT, HEADS, 128), np.float32)
    for ct in range(NCT):
        sel[ct, 2 * ct, 0:64] = 1.0
        sel[ct, 2 * ct + 1, 64:128] = 1.0

    in_maps_b = []
    for c in range(NCORES):
        bi, ic = c // NIC, c % NIC
        in_maps_b.append({
            "a_t": np.ascontiguousarray(AT[bi][:, ic * IC:(ic + 1) * IC]),
            "s_slice": np.ascontiguousarray(S[bi][:, ic * IC:(ic + 1) * IC]),
            "sel": sel,
            "wo": Wo,
        })
    res_b = run_bass_kernel_spmd(_cache["b"], in_maps_b, list(range(NCORES)),
                                 trace=PROFILE["enabled"])
    if PROFILE["enabled"]:
        PROFILE["b_ns"] = res_b.exec_time_ns

    out = np.empty((B, N, DIM), np.float32)
    for c in range(NCORES):
        bi, ic = c // NIC, c % NIC
        out[bi, ic * IC:(ic + 1) * IC, :] = res_b.results[c]["out_rows"]
    return out

